# revision 7
# baseline (speedup 1.0000x reference)
"""Trainium2 Bass kernel for nn_AttentionModel: single-head attention with
vocab-sized input/output projections, tensor-parallel across 8 NeuronCores.

Math (reference):
    Q = x @ Wq + bq ; K = x @ Wk + bk ; V = x @ Wv + bv        [S, E]
    scores = Q @ K^T / sqrt(E)                                  [S, S]
    out = softmax(scores) @ V @ Wo + bo                         [S, VOCAB]

Sharding: vocab dim (50257, padded to 8*6400) split across 8 cores.
  Phase A: per-core partial K^T = Wk_c^T @ x_c^T       -> AllReduce
  Phase Bv: per-core partial V = x_c @ Wv_c            -> AllReduce
  Phase Bq: per-core partial Q = x_c @ Wq_c(scaled)    -> ReduceScatter
            (each core ends with its own 256-query slice of Q)
  Phase C: scores^T = K @ Q_s^T, exp (no max subtraction needed: scores are
           ~N(0,1)), unnormalized ctx^T = V^T @ exp^T, denominators via
           ones-matmul.  ctx^T + denom row -> AllGather
  Phase D: out_c = ctx @ Wo_c, normalized by 1/denom at PSUM eviction.
All matmuls run bf16 inputs with fp32 PSUM accumulation.  1/sqrt(E) is folded
into Wq host-side; Q/K/V biases ride a ones-row in the padded vocab dim.
"""

import sys

if "/opt/trn_rl_repo" not in sys.path:
    sys.path.insert(0, "/opt/trn_rl_repo")

import numpy as np
import ml_dtypes

import concourse.bass as bass
import concourse.tile as tile
from concourse import bacc, mybir
from concourse import bass_utils
from concourse.masks import make_identity

BF16 = mybir.dt.bfloat16
F32 = mybir.dt.float32
NP_BF16 = ml_dtypes.bfloat16


class Cfg:
    def __init__(self, S=2048, E=768, VS=6400, n_cores=8, vocab=50257):
        assert S % 512 == 0 and E % 128 == 0 and VS % 128 == 0
        self.S = S  # full sequence
        self.E = E  # embed dim
        self.VS = VS  # padded vocab rows per core
        self.n_cores = n_cores
        self.vocab = vocab
        self.ST = S // 128  # seq tiles
        self.ET = E // 128  # embed tiles
        self.KT = VS // 128  # contraction (vocab) tiles per core
        self.QS = S // n_cores  # queries per core
        assert self.QS % 128 == 0
        self.QT = self.QS // 128
        self.SC = S // 512  # 512-wide seq chunks (phase A)
        # phase D vocab chunks (over this core's VS output columns)
        self.nch = [(i * 512, min(512, VS - i * 512)) for i in range((VS + 511) // 512)]


FULL = Cfg()


def build_nc(cfg: Cfg):
    S, E, VS = cfg.S, cfg.E, cfg.VS
    ST, ET, KT, QS, QT = cfg.ST, cfg.ET, cfg.KT, cfg.QS, cfg.QT
    RG = [list(range(cfg.n_cores))]

    nc = bacc.Bacc(None, target_bir_lowering=False, num_devices=cfg.n_cores)

    xT = nc.dram_tensor("xT", [VS, S], BF16, kind="ExternalInput")
    wq = nc.dram_tensor("wq", [VS, E], BF16, kind="ExternalInput")
    wk = nc.dram_tensor("wk", [VS, E], BF16, kind="ExternalInput")
    wv = nc.dram_tensor("wv", [VS, E], BF16, kind="ExternalInput")
    wo = nc.dram_tensor("wo", [E, VS], BF16, kind="ExternalInput")
    out = nc.dram_tensor("out", [S, VS], F32, kind="ExternalOutput")

    xT_t = xT.ap().rearrange("(kt p) s -> p kt s", p=128)
    wq_t = wq.ap().rearrange("(kt p) e -> p kt e", p=128)
    wk_t = wk.ap().rearrange("(kt p) e -> p kt e", p=128)
    wv_t = wv.ap().rearrange("(kt p) e -> p kt e", p=128)
    wo_t = wo.ap().rearrange("(et p) v -> p et v", p=128)

    # internal DRAM for collectives
    kt_in = nc.dram_tensor("kt_in", [E, S], F32)
    kt_out = nc.dram_tensor("kt_out", [E, S], F32, addr_space="Shared")
    v_in = nc.dram_tensor("v_in", [S, E], F32)
    v_out = nc.dram_tensor("v_out", [S, E], F32, addr_space="Shared")
    q_in = nc.dram_tensor("q_in", [S, E], F32)
    q_out = nc.dram_tensor("q_out", [QS, E], F32)
    ctx_in = nc.dram_tensor("ctx_in", [E + 1, QS], F32)
    ctx_out = nc.dram_tensor(
        "ctx_out", [cfg.n_cores * (E + 1), QS], F32, addr_space="Shared"
    )

    def load_w_chunked(dst_sb, src_t, ntiles, chunk=10):
        for g0 in range(0, ntiles, chunk):
            g1 = min(g0 + chunk, ntiles)
            nc.sync.dma_start(out=dst_sb[:, g0:g1, :], in_=src_t[:, g0:g1, :])

    with tile.TileContext(nc) as tc:
        xio = tc.alloc_tile_pool(name="xio", bufs=8)
        evict = tc.alloc_tile_pool(name="evict", bufs=4)
        const = tc.alloc_tile_pool(name="const", bufs=1)

        id128 = const.tile([128, 128], F32)
        make_identity(nc, id128)
        ones = const.tile([128, 1], BF16)
        nc.vector.memset(ones, 1.0)

        # ---------------- Phase A: partial K^T = Wk_c^T @ x_c^T ----------------
        wkp = tc.alloc_tile_pool(name="wkp", bufs=1)
        psA = tc.alloc_tile_pool(name="psA", bufs=8, space="PSUM")
        if True:
            wk_sb = wkp.tile([128, KT, E], BF16)
            load_w_chunked(wk_sb, wk_t, KT)
            # prefetch next phase's weights while A computes
            wvp = tc.alloc_tile_pool(name="wvp", bufs=1, side="right")
            wv_sb = wvp.tile([128, KT, E], BF16)
            load_w_chunked(wv_sb, wv_t, KT)

            for sc in range(cfg.SC):
                ps_list = [psA.tile([128, 512], F32, name=f"psa_{sc}_{i}", tag="psa") for i in range(ET)]
                for k in range(KT):
                    xt = xio.tile([128, 512], BF16)
                    nc.sync.dma_start(out=xt, in_=xT_t[:, k, sc * 512 : (sc + 1) * 512])
                    for em in range(ET):
                        nc.tensor.matmul(
                            ps_list[em],
                            lhsT=wk_sb[:, k, em * 128 : (em + 1) * 128],
                            rhs=xt,
                            start=(k == 0),
                            stop=(k == KT - 1),
                        )
                for em in range(ET):
                    st = evict.tile([128, 512], F32)
                    nc.vector.tensor_copy(st, ps_list[em])
                    nc.sync.dma_start(
                        out=kt_in[em * 128 : (em + 1) * 128, sc * 512 : (sc + 1) * 512],
                        in_=st,
                    )
        psA.release()
        wkp.release()
        nc.gpsimd.collective_compute(
            "AllReduce",
            mybir.AluOpType.add,
            replica_groups=RG,
            ins=[kt_in.ap().opt()],
            outs=[kt_out.ap().opt()],
        )

        # ---------------- Phase Bv: partial V = x_c @ Wv_c ----------------
        def phase_b(w_sb, dst_dram):
            psB = tc.alloc_tile_pool(name="psB", bufs=4, space="PSUM")
            if True:
                for mg in range(ST // 2):
                    ps2 = [psB.tile([128, E], F32, name=f"psb_{mg}_{i}", tag="psb") for i in range(2)]
                    for k in range(KT):
                        xt2 = xio.tile([128, 256], BF16)
                        nc.sync.dma_start(
                            out=xt2, in_=xT_t[:, k, mg * 256 : (mg + 1) * 256]
                        )
                        for m2 in range(2):
                            for c0 in range(0, E, 512):
                                c1 = min(c0 + 512, E)
                                nc.tensor.matmul(
                                    ps2[m2][:, c0:c1],
                                    lhsT=xt2[:, m2 * 128 : (m2 + 1) * 128],
                                    rhs=w_sb[:, k, c0:c1],
                                    start=(k == 0),
                                    stop=(k == KT - 1),
                                )
                    for m2 in range(2):
                        st = evict.tile([128, E], F32)
                        nc.vector.tensor_copy(st, ps2[m2])
                        r0 = (mg * 2 + m2) * 128
                        nc.sync.dma_start(out=dst_dram[r0 : r0 + 128, :], in_=st)
            psB.release()

        phase_b(wv_sb, v_in)
        # free wv, prefetch wq
        wqp = tc.alloc_tile_pool(name="wqp", bufs=1)
        wq_sb = wqp.tile([128, KT, E], BF16)
        load_w_chunked(wq_sb, wq_t, KT)
        wvp.release()

        nc.gpsimd.collective_compute(
            "AllReduce",
            mybir.AluOpType.add,
            replica_groups=RG,
            ins=[v_in.ap().opt()],
            outs=[v_out.ap().opt()],
        )

        # ---------------- Phase Bq: partial Q = x_c @ Wq_c ----------------
        # prefetch wo during Bq
        wop = tc.alloc_tile_pool(name="wop", bufs=1, side="right")
        wo_sb = wop.tile([128, ET, VS], BF16)
        load_w_chunked(wo_sb, wo_t, ET, chunk=1)

        phase_b(wq_sb, q_in)
        wqp.release()

        nc.gpsimd.collective_compute(
            "ReduceScatter",
            mybir.AluOpType.add,
            replica_groups=RG,
            ins=[q_in.ap().opt()],
            outs=[q_out.ap().opt()],
        )

        # ---------------- Phase C: attention on this core's query slice ----------------
        ap_ = tc.alloc_tile_pool(name="attnp", bufs=1)
        cstage = tc.alloc_tile_pool(name="cstage", bufs=4)
        psC = tc.alloc_tile_pool(name="psC", bufs=2, space="PSUM")
        if True:
            # K^T -> bf16 (chunked loads to keep staging slots small)
            kt_sb = ap_.tile([128, ET, S], BF16)
            for et in range(ET):
                for sc4 in range(S // 512):
                    stg = cstage.tile([128, 512], F32)
                    nc.sync.dma_start(
                        out=stg,
                        in_=kt_out[
                            et * 128 : (et + 1) * 128, sc4 * 512 : (sc4 + 1) * 512
                        ],
                    )
                    nc.vector.tensor_copy(
                        kt_sb[:, et, sc4 * 512 : (sc4 + 1) * 512], stg
                    )
            # V -> bf16
            v_sb = ap_.tile([128, ST, E], BF16)
            for st_i in range(ST):
                stgv = cstage.tile([128, E], F32)
                nc.sync.dma_start(
                    out=stgv, in_=v_out[st_i * 128 : (st_i + 1) * 128, :]
                )
                nc.vector.tensor_copy(v_sb[:, st_i, :], stgv)
            # Q_s -> transpose to [E, QS] bf16
            qT_sb = ap_.tile([128, ET, QS], BF16)
            for qt in range(QT):
                qstg = cstage.tile([128, E], F32)
                nc.sync.dma_start(
                    out=qstg, in_=q_out[qt * 128 : (qt + 1) * 128, :]
                )
                for et in range(ET):
                    ps_t = psC.tile([128, 128], F32)
                    nc.tensor.transpose(
                        ps_t, qstg[:, et * 128 : (et + 1) * 128], id128
                    )
                    nc.vector.tensor_copy(
                        qT_sb[:, et, qt * 128 : (qt + 1) * 128], ps_t
                    )
            # scores^T tiles + exp
            expT_sb = ap_.tile([128, ST, QS], BF16)
            for mk in range(ST):
                ps_s = psC.tile([128, QS], F32)
                for et in range(ET):
                    nc.tensor.matmul(
                        ps_s,
                        lhsT=kt_sb[:, et, mk * 128 : (mk + 1) * 128],
                        rhs=qT_sb[:, et, :],
                        start=(et == 0),
                        stop=(et == ET - 1),
                    )
                nc.scalar.activation(
                    out=expT_sb[:, mk, :], in_=ps_s, func=mybir.ActivationFunctionType.Exp
                )
            # denominators: column sums of exp^T via ones-matmul
            ps_d = psC.tile([1, QS], F32)
            for mk in range(ST):
                nc.tensor.matmul(
                    ps_d,
                    lhsT=ones,
                    rhs=expT_sb[:, mk, :],
                    start=(mk == 0),
                    stop=(mk == ST - 1),
                )
            dstg = cstage.tile([1, QS], F32)
            nc.vector.tensor_copy(dstg, ps_d)
            nc.sync.dma_start(out=ctx_in[E : E + 1, :], in_=dstg)
            # unnormalized ctx^T = V^T @ exp^T
            for et in range(ET):
                ps_c = psC.tile([128, QS], F32)
                for mk in range(ST):
                    nc.tensor.matmul(
                        ps_c,
                        lhsT=v_sb[:, mk, et * 128 : (et + 1) * 128],
                        rhs=expT_sb[:, mk, :],
                        start=(mk == 0),
                        stop=(mk == ST - 1),
                    )
                cstg = cstage.tile([128, QS], F32)
                nc.vector.tensor_copy(cstg, ps_c)
                nc.sync.dma_start(
                    out=ctx_in[et * 128 : (et + 1) * 128, :], in_=cstg
                )
        psC.release()
        cstage.release()
        ap_.release()
        nc.gpsimd.collective_compute(
            "AllGather",
            mybir.AluOpType.bypass,
            replica_groups=RG,
            ins=[ctx_in.ap().opt()],
            outs=[ctx_out.ap().opt()],
        )

        # ---------------- Phase D: out_c = ctx @ Wo_c / denom ----------------
        blk = E + 1
        ctxp = tc.alloc_tile_pool(name="ctxp", bufs=4)
        osb = tc.alloc_tile_pool(name="osb", bufs=8)
        psD = tc.alloc_tile_pool(name="psD", bufs=8, space="PSUM")
        if True:
            for ms in range(ST):
                c_blk = ms // QT
                h = ms % QT
                cstg = ctxp.tile([128, ET, 128], F32)
                for et in range(ET):
                    nc.sync.dma_start(
                        out=cstg[:, et, :],
                        in_=ctx_out[
                            blk * c_blk + et * 128 : blk * c_blk + (et + 1) * 128,
                            h * 128 : (h + 1) * 128,
                        ],
                    )
                ctxT_bf = ctxp.tile([128, ET, 128], BF16)
                nc.vector.tensor_copy(ctxT_bf, cstg)
                dstg2 = ctxp.tile([128, 1], F32)
                nc.sync.dma_start(
                    out=dstg2,
                    in_=ctx_out[
                        blk * c_blk + E : blk * c_blk + E + 1,
                        h * 128 : (h + 1) * 128,
                    ].rearrange("one q -> q one"),
                )
                recip = ctxp.tile([128, 1], F32)
                nc.vector.reciprocal(recip, dstg2)
                for n0, nsz in cfg.nch:
                    ps_o = psD.tile([128, 512], F32)
                    for et in range(ET):
                        nc.tensor.matmul(
                            ps_o[:, :nsz],
                            lhsT=ctxT_bf[:, et, :],
                            rhs=wo_sb[:, et, n0 : n0 + nsz],
                            start=(et == 0),
                            stop=(et == ET - 1),
                        )
                    ost = osb.tile([128, 512], F32)
                    nc.vector.tensor_scalar_mul(ost[:, :nsz], ps_o[:, :nsz], recip)
                    nc.sync.dma_start(
                        out=out[ms * 128 : (ms + 1) * 128, n0 : n0 + nsz],
                        in_=ost[:, :nsz],
                    )
        psD.release()
        osb.release()
        ctxp.release()
        wop.release()
        const.release()
        evict.release()
        xio.release()

    nc.compile()
    return nc


def _shard_bounds(cfg: Cfg):
    base = cfg.vocab // cfg.n_cores
    rem = cfg.vocab % cfg.n_cores
    sizes = [base + (1 if c < rem else 0) for c in range(cfg.n_cores)]
    starts = [sum(sizes[:c]) for c in range(cfg.n_cores)]
    return starts, sizes


def prepare_inputs(cfg: Cfg, x, Wq, bq, Wk, bk, Wv, bv, Wo):
    """Host-side shard/pad/cast. Returns in_maps for run_bass_kernel_spmd."""
    S, E, VS, N = cfg.S, cfg.E, cfg.VS, cfg.n_cores
    inv = np.float32(1.0 / np.sqrt(E))
    xT = np.ascontiguousarray(x.reshape(S, -1).T.astype(np.float32)).astype(NP_BF16)
    Wq_s = (Wq.astype(np.float32) * inv).astype(NP_BF16)
    Wk_s = Wk.astype(np.float32).astype(NP_BF16)
    Wv_s = Wv.astype(np.float32).astype(NP_BF16)
    Wo_s = Wo.astype(np.float32).astype(NP_BF16)
    bq_s = (bq.astype(np.float32) * inv / N).astype(np.float32)
    bk_s = (bk.astype(np.float32) / N).astype(np.float32)
    bv_s = (bv.astype(np.float32) / N).astype(np.float32)

    starts, sizes = _shard_bounds(cfg)
    in_maps = []
    for c in range(N):
        s0, rv = starts[c], sizes[c]
        assert rv <= VS - 1, "need a free padded row for the bias/ones row"
        xs = np.zeros((VS, S), dtype=NP_BF16)
        xs[:rv] = xT[s0 : s0 + rv]
        xs[VS - 1] = NP_BF16(1.0)
        wqc = np.zeros((VS, E), dtype=NP_BF16)
        wqc[:rv] = Wq_s[s0 : s0 + rv]
        wqc[VS - 1] = bq_s.astype(NP_BF16)
        wkc = np.zeros((VS, E), dtype=NP_BF16)
        wkc[:rv] = Wk_s[s0 : s0 + rv]
        wkc[VS - 1] = bk_s.astype(NP_BF16)
        wvc = np.zeros((VS, E), dtype=NP_BF16)
        wvc[:rv] = Wv_s[s0 : s0 + rv]
        wvc[VS - 1] = bv_s.astype(NP_BF16)
        woc = np.zeros((E, VS), dtype=NP_BF16)
        woc[:, :rv] = Wo_s[:, s0 : s0 + rv]
        in_maps.append({"xT": xs, "wq": wqc, "wk": wkc, "wv": wvc, "wo": woc})
    return in_maps


def assemble_output(cfg: Cfg, results, bo):
    starts, sizes = _shard_bounds(cfg)
    parts = [results[c]["out"][:, : sizes[c]] for c in range(cfg.n_cores)]
    full = np.concatenate(parts, axis=1)
    full = full + bo.astype(np.float32)[None, :]
    return full[None].astype(np.float32)


_NC_CACHE = {}


def _get_nc(cfg: Cfg):
    key = (cfg.S, cfg.E, cfg.VS, cfg.n_cores)
    if key not in _NC_CACHE:
        _NC_CACHE[key] = build_nc(cfg)
    return _NC_CACHE[key]


def kernel(x, Wq, bq, Wk, bk, Wv, bv, Wo, bo):
    cfg = FULL
    x = np.asarray(x)
    in_maps = prepare_inputs(
        cfg,
        x,
        np.asarray(Wq),
        np.asarray(bq),
        np.asarray(Wk),
        np.asarray(bk),
        np.asarray(Wv),
        np.asarray(bv),
        np.asarray(Wo),
    )
    nc = _get_nc(cfg)
    res = bass_utils.run_bass_kernel_spmd(
        nc, in_maps, core_ids=list(range(cfg.n_cores))
    )
    return assemble_output(cfg, res.results, np.asarray(bo))


# revision 11
# speedup vs baseline: 79.4432x; 79.4432x over previous
"""Trainium2 Bass kernel for nn_AttentionModel: single-head attention with
vocab-sized input/output projections, tensor-parallel across 8 NeuronCores.

Math (reference):
    Q = x @ Wq + bq ; K = x @ Wk + bk ; V = x @ Wv + bv        [S, E]
    scores = Q @ K^T / sqrt(E)                                  [S, S]
    out = softmax(scores) @ V @ Wo + bo                         [S, VOCAB]

Sharding: vocab dim (50257, padded to 8*6400) split across 8 cores.
  Phase A: per-core partial K^T = Wk_c^T @ x_c^T       -> AllReduce
  Phase Bv: per-core partial V = x_c @ Wv_c            -> AllReduce
  Phase Bq: per-core partial Q = x_c @ Wq_c(scaled)    -> ReduceScatter
            (each core ends with its own 256-query slice of Q)
  Phase C: scores^T = K @ Q_s^T, exp (no max subtraction needed: scores are
           ~N(0,1)), unnormalized ctx^T = V^T @ exp^T, denominators via
           ones-matmul.  ctx^T + denom row -> AllGather
  Phase D: out_c = ctx @ Wo_c, normalized by 1/denom at PSUM eviction.
All matmuls run bf16 inputs with fp32 PSUM accumulation.  1/sqrt(E) is folded
into Wq host-side; Q/K/V biases ride a ones-row in the padded vocab dim.
"""

import sys

if "/opt/trn_rl_repo" not in sys.path:
    sys.path.insert(0, "/opt/trn_rl_repo")

import numpy as np
import ml_dtypes

import concourse.bass as bass
import concourse.tile as tile
from concourse import bacc, mybir
from concourse import bass_utils
from concourse.masks import make_identity

BF16 = mybir.dt.bfloat16
F32 = mybir.dt.float32
NP_BF16 = ml_dtypes.bfloat16


class Cfg:
    def __init__(self, S=2048, E=768, VS=6400, n_cores=8, vocab=50257):
        assert S % 512 == 0 and E % 128 == 0 and VS % 128 == 0
        self.S = S  # full sequence
        self.E = E  # embed dim
        self.VS = VS  # padded vocab rows per core
        self.n_cores = n_cores
        self.vocab = vocab
        self.ST = S // 128  # seq tiles
        self.ET = E // 128  # embed tiles
        self.KT = VS // 128  # contraction (vocab) tiles per core
        self.QS = S // n_cores  # queries per core
        assert self.QS % 128 == 0
        self.QT = self.QS // 128
        self.SC = S // 512  # 512-wide seq chunks (phase A)
        # phase D vocab chunks (over this core's VS output columns)
        self.nch = [(i * 512, min(512, VS - i * 512)) for i in range((VS + 511) // 512)]


FULL = Cfg()


def build_nc(cfg: Cfg, reps: int = 1, emulate_cc: bool = False):
    S, E, VS = cfg.S, cfg.E, cfg.VS
    ST, ET, KT, QS, QT = cfg.ST, cfg.ET, cfg.KT, cfg.QS, cfg.QT
    RG = [list(range(cfg.n_cores))]

    nc = bacc.Bacc(None, target_bir_lowering=False, num_devices=cfg.n_cores)

    xT = nc.dram_tensor("xT", [VS, S], BF16, kind="ExternalInput")
    wq = nc.dram_tensor("wq", [VS, E], BF16, kind="ExternalInput")
    wk = nc.dram_tensor("wk", [VS, E], BF16, kind="ExternalInput")
    wv = nc.dram_tensor("wv", [VS, E], BF16, kind="ExternalInput")
    wo = nc.dram_tensor("wo", [E, VS], BF16, kind="ExternalInput")
    out = nc.dram_tensor("out", [S, VS], F32, kind="ExternalOutput")

    xT_t = xT.ap().rearrange("(kt p) s -> p kt s", p=128)
    wq_t = wq.ap().rearrange("(kt p) e -> p kt e", p=128)
    wk_t = wk.ap().rearrange("(kt p) e -> p kt e", p=128)
    wv_t = wv.ap().rearrange("(kt p) e -> p kt e", p=128)
    wo_t = wo.ap().rearrange("(et p) v -> p et v", p=128)

    # internal DRAM for collectives
    kt_in = nc.dram_tensor("kt_in", [E, S], BF16)
    kt_out = nc.dram_tensor("kt_out", [E, S], BF16, addr_space="Shared")
    v_in = nc.dram_tensor("v_in", [S, E], BF16)
    v_out = nc.dram_tensor("v_out", [S, E], BF16, addr_space="Shared")
    q_in = nc.dram_tensor("q_in", [S, E], BF16)
    q_out = nc.dram_tensor("q_out", [QS, E], BF16)
    ctx_in = nc.dram_tensor("ctx_in", [E + 2, QS], BF16)
    ctx_out = nc.dram_tensor(
        "ctx_out", [cfg.n_cores * (E + 2), QS], BF16, addr_space="Shared"
    )

    def load_w_chunked(dst_sb, src_t, ntiles, chunk=10):
        for g0 in range(0, ntiles, chunk):
            g1 = min(g0 + chunk, ntiles)
            nc.sync.dma_start(out=dst_sb[:, g0:g1, :], in_=src_t[:, g0:g1, :])

    def do_cc(kind, in_t, out_t):
        if not emulate_cc:
            op = (
                mybir.AluOpType.bypass
                if kind == "AllGather"
                else mybir.AluOpType.add
            )
            nc.gpsimd.collective_compute(
                kind,
                op,
                replica_groups=RG,
                ins=[in_t.ap().opt()],
                outs=[out_t.ap().opt()],
            )
            return
        # single-core emulation with plain DMA (preserves deps for TimelineSim)
        ish, osh = in_t.shape, out_t.shape
        if kind == "AllReduce":
            nc.sync.dma_start(out=out_t[:, :], in_=in_t[:, :])
        elif kind == "ReduceScatter":
            nc.sync.dma_start(out=out_t[:, :], in_=in_t[0 : osh[0], :])
        elif kind == "AllGather":
            for c in range(cfg.n_cores):
                nc.sync.dma_start(
                    out=out_t[c * ish[0] : (c + 1) * ish[0], :], in_=in_t[:, :]
                )

    with tile.TileContext(nc) as tc:
        xio = tc.alloc_tile_pool(name="xio", bufs=8)
        evict = tc.alloc_tile_pool(name="evict", bufs=4)
        const = tc.alloc_tile_pool(name="const", bufs=1)

        id128 = const.tile([128, 128], BF16)
        make_identity(nc, id128)
        ones = const.tile([128, 1], BF16)
        nc.vector.memset(ones, 1.0)

        def phase_b(w_sb, dst_dram):
            psB = tc.alloc_tile_pool(name="psB", bufs=4, space="PSUM")
            for mg in range(ST // 2):
                ps2 = [
                    psB.tile([128, E], F32, name=f"psb_{mg}_{i}", tag="psb")
                    for i in range(2)
                ]
                for k in range(KT):
                    xt2 = xio.tile([128, 256], BF16)
                    nc.sync.dma_start(
                        out=xt2, in_=xT_t[:, k, mg * 256 : (mg + 1) * 256]
                    )
                    for m2 in range(2):
                        for c0 in range(0, E, 512):
                            c1 = min(c0 + 512, E)
                            nc.tensor.matmul(
                                ps2[m2][:, c0:c1],
                                lhsT=xt2[:, m2 * 128 : (m2 + 1) * 128],
                                rhs=w_sb[:, k, c0:c1],
                                start=(k == 0),
                                stop=(k == KT - 1),
                            )
                for m2 in range(2):
                    st = evict.tile([128, E], BF16)
                    nc.vector.tensor_copy(st, ps2[m2])
                    r0 = (mg * 2 + m2) * 128
                    nc.sync.dma_start(out=dst_dram[r0 : r0 + 128, :], in_=st)
            psB.release()

        for rep in range(reps):
            # ------------- Phase A: partial K^T = Wk_c^T @ x_c^T -------------
            wkp = tc.alloc_tile_pool(name="wkp", bufs=1)
            psA = tc.alloc_tile_pool(name="psA", bufs=8, space="PSUM")
            wk_sb = wkp.tile([128, KT, E], BF16)
            load_w_chunked(wk_sb, wk_t, KT)
            # prefetch next phase's weights while A computes
            wqp = tc.alloc_tile_pool(name="wqp", bufs=1, side="right")
            wq_sb = wqp.tile([128, KT, E], BF16)
            load_w_chunked(wq_sb, wq_t, KT)

            for sc in range(cfg.SC):
                ps_list = [
                    psA.tile([128, 512], F32, name=f"psa_{sc}_{i}", tag="psa")
                    for i in range(ET)
                ]
                for k in range(KT):
                    xt = xio.tile([128, 512], BF16)
                    nc.sync.dma_start(
                        out=xt, in_=xT_t[:, k, sc * 512 : (sc + 1) * 512]
                    )
                    for em in range(ET):
                        nc.tensor.matmul(
                            ps_list[em],
                            lhsT=wk_sb[:, k, em * 128 : (em + 1) * 128],
                            rhs=xt,
                            start=(k == 0),
                            stop=(k == KT - 1),
                        )
                for em in range(ET):
                    st = evict.tile([128, 512], BF16)
                    nc.vector.tensor_copy(st, ps_list[em])
                    nc.sync.dma_start(
                        out=kt_in[
                            em * 128 : (em + 1) * 128, sc * 512 : (sc + 1) * 512
                        ],
                        in_=st,
                    )
            psA.release()
            wkp.release()
            do_cc("AllReduce", kt_in, kt_out)

            # ------------- Phase Bq: partial Q = x_c @ Wq_c -------------
            # prefetch wv during Bq
            wvp = tc.alloc_tile_pool(name="wvp", bufs=1)
            wv_sb = wvp.tile([128, KT, E], BF16)
            load_w_chunked(wv_sb, wv_t, KT)

            phase_b(wq_sb, q_in)
            wqp.release()
            do_cc("ReduceScatter", q_in, q_out)

            # ------------- Phase Bv: partial V = x_c @ Wv_c -------------
            # prefetch wo during Bv
            wop = tc.alloc_tile_pool(name="wop", bufs=1, side="right")
            wo_sb = wop.tile([128, ET, VS], BF16)
            load_w_chunked(wo_sb, wo_t, ET, chunk=1)

            phase_b(wv_sb, v_in)
            wvp.release()
            do_cc("AllReduce", v_in, v_out)

            # ------------- Phase C: attention on this core's query slice -------------
            ap_ = tc.alloc_tile_pool(name="attnp", bufs=1)
            cstage = tc.alloc_tile_pool(name="cstage", bufs=4)
            psC = tc.alloc_tile_pool(name="psC", bufs=2, space="PSUM")
            # K^T and V arrive as bf16 from the collectives: load directly
            kt_sb = ap_.tile([128, ET, S], BF16)
            for et in range(ET):
                nc.sync.dma_start(
                    out=kt_sb[:, et, :], in_=kt_out[et * 128 : (et + 1) * 128, :]
                )
            v_sb = ap_.tile([128, ST, E], BF16)
            for st_i in range(ST):
                nc.sync.dma_start(
                    out=v_sb[:, st_i, :], in_=v_out[st_i * 128 : (st_i + 1) * 128, :]
                )
            # Q_s -> transpose to [E, QS] bf16
            qT_sb = ap_.tile([128, ET, QS], BF16)
            for qt in range(QT):
                qstg = cstage.tile([128, E], BF16)
                nc.sync.dma_start(
                    out=qstg, in_=q_out[qt * 128 : (qt + 1) * 128, :]
                )
                for et in range(ET):
                    ps_t = psC.tile([128, 128], BF16)
                    nc.tensor.transpose(
                        ps_t, qstg[:, et * 128 : (et + 1) * 128], id128
                    )
                    nc.vector.tensor_copy(
                        qT_sb[:, et, qt * 128 : (qt + 1) * 128], ps_t
                    )
            # scores^T tiles + exp
            expT_sb = ap_.tile([128, ST, QS], BF16)
            for mk in range(ST):
                ps_s = psC.tile([128, QS], F32)
                for et in range(ET):
                    nc.tensor.matmul(
                        ps_s,
                        lhsT=kt_sb[:, et, mk * 128 : (mk + 1) * 128],
                        rhs=qT_sb[:, et, :],
                        start=(et == 0),
                        stop=(et == ET - 1),
                    )
                nc.scalar.activation(
                    out=expT_sb[:, mk, :],
                    in_=ps_s,
                    func=mybir.ActivationFunctionType.Exp,
                )
            # denominators: column sums of exp^T via ones-matmul
            ps_d = psC.tile([1, QS], F32)
            for mk in range(ST):
                nc.tensor.matmul(
                    ps_d,
                    lhsT=ones,
                    rhs=expT_sb[:, mk, :],
                    start=(mk == 0),
                    stop=(mk == ST - 1),
                )
            dhi = cstage.tile([1, QS], BF16)
            nc.vector.tensor_copy(dhi, ps_d)
            dhi_f = cstage.tile([1, QS], F32)
            nc.vector.tensor_copy(dhi_f, dhi)
            dlo_f = cstage.tile([1, QS], F32)
            nc.vector.tensor_sub(dlo_f, ps_d, dhi_f)
            dlo = cstage.tile([1, QS], BF16)
            nc.vector.tensor_copy(dlo, dlo_f)
            nc.sync.dma_start(out=ctx_in[E : E + 1, :], in_=dhi)
            nc.sync.dma_start(out=ctx_in[E + 1 : E + 2, :], in_=dlo)
            # unnormalized ctx^T = V^T @ exp^T
            for et in range(ET):
                ps_c = psC.tile([128, QS], F32)
                for mk in range(ST):
                    nc.tensor.matmul(
                        ps_c,
                        lhsT=v_sb[:, mk, et * 128 : (et + 1) * 128],
                        rhs=expT_sb[:, mk, :],
                        start=(mk == 0),
                        stop=(mk == ST - 1),
                    )
                cstg = cstage.tile([128, QS], BF16)
                nc.vector.tensor_copy(cstg, ps_c)
                nc.sync.dma_start(
                    out=ctx_in[et * 128 : (et + 1) * 128, :], in_=cstg
                )
            psC.release()
            cstage.release()
            ap_.release()
            do_cc("AllGather", ctx_in, ctx_out)

            # ------------- Phase D: out_c = ctx @ Wo_c / denom -------------
            blk = E + 2
            ctxp = tc.alloc_tile_pool(name="ctxp", bufs=4)
            osb = tc.alloc_tile_pool(name="osb", bufs=8)
            psD = tc.alloc_tile_pool(name="psD", bufs=8, space="PSUM")
            for ms in range(ST):
                c_blk = ms // QT
                h = ms % QT
                ctxT_bf = ctxp.tile([128, ET, 128], BF16)
                for et in range(ET):
                    nc.sync.dma_start(
                        out=ctxT_bf[:, et, :],
                        in_=ctx_out[
                            blk * c_blk + et * 128 : blk * c_blk + (et + 1) * 128,
                            h * 128 : (h + 1) * 128,
                        ],
                    )
                dhi2 = ctxp.tile([128, 1], BF16)
                nc.sync.dma_start(
                    out=dhi2,
                    in_=ctx_out[
                        blk * c_blk + E : blk * c_blk + E + 1,
                        h * 128 : (h + 1) * 128,
                    ].rearrange("one q -> q one"),
                )
                dlo2 = ctxp.tile([128, 1], BF16)
                nc.sync.dma_start(
                    out=dlo2,
                    in_=ctx_out[
                        blk * c_blk + E + 1 : blk * c_blk + E + 2,
                        h * 128 : (h + 1) * 128,
                    ].rearrange("one q -> q one"),
                )
                dsum = ctxp.tile([128, 1], F32)
                nc.vector.tensor_add(dsum, dhi2, dlo2)
                recip = ctxp.tile([128, 1], F32)
                nc.vector.reciprocal(recip, dsum)
                for n0, nsz in cfg.nch:
                    ps_o = psD.tile([128, 512], F32)
                    for et in range(ET):
                        nc.tensor.matmul(
                            ps_o[:, :nsz],
                            lhsT=ctxT_bf[:, et, :],
                            rhs=wo_sb[:, et, n0 : n0 + nsz],
                            start=(et == 0),
                            stop=(et == ET - 1),
                        )
                    ost = osb.tile([128, 512], F32)
                    nc.vector.tensor_scalar_mul(ost[:, :nsz], ps_o[:, :nsz], recip)
                    nc.sync.dma_start(
                        out=out[ms * 128 : (ms + 1) * 128, n0 : n0 + nsz],
                        in_=ost[:, :nsz],
                    )
            psD.release()
            osb.release()
            ctxp.release()
            wop.release()

        const.release()
        evict.release()
        xio.release()

    nc.compile()
    return nc


def _shard_bounds(cfg: Cfg):
    base = cfg.vocab // cfg.n_cores
    rem = cfg.vocab % cfg.n_cores
    sizes = [base + (1 if c < rem else 0) for c in range(cfg.n_cores)]
    starts = [sum(sizes[:c]) for c in range(cfg.n_cores)]
    return starts, sizes


def prepare_inputs(cfg: Cfg, x, Wq, bq, Wk, bk, Wv, bv, Wo):
    """Host-side shard/pad/cast. Returns in_maps for run_bass_kernel_spmd."""
    S, E, VS, N = cfg.S, cfg.E, cfg.VS, cfg.n_cores
    inv = np.float32(1.0 / np.sqrt(E))
    xT = np.ascontiguousarray(x.reshape(S, -1).T.astype(np.float32)).astype(NP_BF16)
    Wq_s = (Wq.astype(np.float32) * inv).astype(NP_BF16)
    Wk_s = Wk.astype(np.float32).astype(NP_BF16)
    Wv_s = Wv.astype(np.float32).astype(NP_BF16)
    Wo_s = Wo.astype(np.float32).astype(NP_BF16)
    bq_s = (bq.astype(np.float32) * inv / N).astype(np.float32)
    bk_s = (bk.astype(np.float32) / N).astype(np.float32)
    bv_s = (bv.astype(np.float32) / N).astype(np.float32)

    starts, sizes = _shard_bounds(cfg)
    in_maps = []
    for c in range(N):
        s0, rv = starts[c], sizes[c]
        assert rv <= VS - 1, "need a free padded row for the bias/ones row"
        xs = np.zeros((VS, S), dtype=NP_BF16)
        xs[:rv] = xT[s0 : s0 + rv]
        xs[VS - 1] = NP_BF16(1.0)
        wqc = np.zeros((VS, E), dtype=NP_BF16)
        wqc[:rv] = Wq_s[s0 : s0 + rv]
        wqc[VS - 1] = bq_s.astype(NP_BF16)
        wkc = np.zeros((VS, E), dtype=NP_BF16)
        wkc[:rv] = Wk_s[s0 : s0 + rv]
        wkc[VS - 1] = bk_s.astype(NP_BF16)
        wvc = np.zeros((VS, E), dtype=NP_BF16)
        wvc[:rv] = Wv_s[s0 : s0 + rv]
        wvc[VS - 1] = bv_s.astype(NP_BF16)
        woc = np.zeros((E, VS), dtype=NP_BF16)
        woc[:, :rv] = Wo_s[:, s0 : s0 + rv]
        in_maps.append({"xT": xs, "wq": wqc, "wk": wkc, "wv": wvc, "wo": woc})
    return in_maps


def assemble_output(cfg: Cfg, results, bo):
    starts, sizes = _shard_bounds(cfg)
    parts = [results[c]["out"][:, : sizes[c]] for c in range(cfg.n_cores)]
    full = np.concatenate(parts, axis=1)
    full = full + bo.astype(np.float32)[None, :]
    return full[None].astype(np.float32)


_NC_CACHE = {}


def _get_nc(cfg: Cfg):
    key = (cfg.S, cfg.E, cfg.VS, cfg.n_cores)
    if key not in _NC_CACHE:
        _NC_CACHE[key] = build_nc(cfg)
    return _NC_CACHE[key]


def kernel(x, Wq, bq, Wk, bk, Wv, bv, Wo, bo):
    cfg = FULL
    x = np.asarray(x)
    in_maps = prepare_inputs(
        cfg,
        x,
        np.asarray(Wq),
        np.asarray(bq),
        np.asarray(Wk),
        np.asarray(bk),
        np.asarray(Wv),
        np.asarray(bv),
        np.asarray(Wo),
    )
    nc = _get_nc(cfg)
    res = bass_utils.run_bass_kernel_spmd(
        nc, in_maps, core_ids=list(range(cfg.n_cores))
    )
    return assemble_output(cfg, res.results, np.asarray(bo))
